# revision 3
# baseline (speedup 1.0000x reference)
"""Causal self-attention with RoPE on 8 Trainium2 NeuronCores (Bass/Tile).

Sharding: 8 cores = 2 batch elements x 4 head-groups (4 heads each), no
collectives. Each core computes QKV for its heads from a host-pretransposed
x^T, applies RoPE, runs causally-trimmed flash-style attention, and emits a
partial output projection against its w_proj row-slice; the host sums 4
partials per batch element.

Key design points:
- bf16 x / qkv weights (full PE rate, ~1e-3 end-to-end error); fp32r
  elsewhere.
- Consolidated DMAs: one descriptor-dense DMA per x^T q-chunk, one merged
  [qkv] weight tensor, whole-table cos/sin loads. The cost model charges
  ~625ns of exclusive HWDGE time per DMA, so DMA count matters.
- PE warmup matmuls on the mask constants ramp the tensor engine to full
  clock while the first real DMAs land.
- Zero on-device transposes: x^T comes from the host; Q^T/K^T are computed
  d-major (2 heads packed per 128 partitions), V is computed t-major.
- rotate_half is a PE matmul against a constant +-1 permutation; the rot
  matmul of chunk i is emitted after the QKV matmuls of chunk i+1 so the PE
  never stalls on the PSUM->SBUF copy.
- S^T blocks [k-tile, 512-wide q-chunk] are causally trimmed; the diagonal
  triangle mask (only ever 128 columns wide) is accumulated into PSUM by a
  bf16 identity-matmul after the S matmul.
- exp on ScalarE reads PSUM directly (scale=1/sqrt(hd) folded in); V~ has a
  ones column so the PV matmul accumulates the softmax denominator for free.
- Normalization: single-partition reciprocal + gpsimd partition_broadcast
  (no DMAs in the chain); the combine add and normalize muls run on the
  otherwise-idle GPSIMD where possible.
- Projection is interleaved into pair-1 attention with a lag so its matmuls
  never head-block the PE queue.
"""
import os

import numpy as np

import concourse.bass as bass
import concourse.mybir as mybir
import concourse.tile as tile
from concourse import bacc
from concourse.bass_utils import run_bass_kernel_spmd

# Problem shape (hardcoded per harness contract).
B, T, C, NH = 2, 2048, 1024, 16
HD = C // NH          # 64
HPC = NH // 4         # 4 heads per core
N_CORES = 8
ROPE_BASE = 10000.0
NEG = -1.0e30

F32 = mybir.dt.float32
BF16 = mybir.dt.bfloat16
USE_F32R = os.environ.get("KERNEL_F32R", "1") == "1"
F32R = mybir.dt.float32r if USE_F32R else mybir.dt.float32
EDT_BF16 = os.environ.get("KERNEL_EDT", "bf16") == "bf16"
XDT_BF16 = os.environ.get("KERNEL_XDT", "bf16") == "bf16"

_CACHE = {}


def _rope_tables_T():
    """cos/sin tables transposed to [HD, T], duplicated to 128 partitions
    (two 64-row head blocks), with the rotate-half sign folded into sin."""
    inv_freq = 1.0 / (ROPE_BASE ** (np.arange(0, HD, 2, dtype=np.float32) / HD))
    t = np.arange(T, dtype=np.float32)
    freqs = np.outer(t, inv_freq).astype(np.float32)      # [T, 32]
    emb = np.concatenate([freqs, freqs], axis=-1)         # [T, 64]
    cosT = np.cos(emb).T.astype(np.float32)               # [64, T]
    sinT = np.sin(emb).T.astype(np.float32)
    cos2 = np.concatenate([cosT, cosT], axis=0)           # [128, T]
    sin2 = np.concatenate([sinT, sinT], axis=0)
    return np.ascontiguousarray(cos2), np.ascontiguousarray(sin2)


def _rot_matrix():
    """rot128 so that (rot128.T @ qT) = rotate_half(q)^T per 64-row head
    block: out[d] = -in[d+32] for d<32, in[d-32] for d>=32."""
    r = np.zeros((64, 64), dtype=np.float32)
    for d in range(32):
        r[d, d + 32] = -1.0
        r[d + 32, d] = 1.0
    z = np.zeros_like(r)
    rot = np.block([[r, z], [z, r]])          # [128, 128]
    return np.ascontiguousarray(rot.T)        # lhsT layout



def _cmask_tile():
    """[128, 256] bf16: identity in cols 0-127, causal triangle (0 / NEG)
    in cols 128-255."""
    import ml_dtypes
    ident = np.eye(128, dtype=np.float32)
    k_l = np.arange(128)[:, None]
    q_l = np.arange(128)[None, :]
    tri = np.where(q_l >= k_l, 0.0, NEG)
    return np.concatenate([ident, tri], axis=1).astype(ml_dtypes.bfloat16)


def build_nc():
    nc = bacc.Bacc(None, target_bir_lowering=False)

    XDT = BF16 if XDT_BF16 else F32R
    EDT = BF16 if EDT_BF16 else F32R
    xT = nc.dram_tensor("xT", [C, T], XDT, kind="ExternalInput")
    w_all = nc.dram_tensor("w_all", [128, C // 128, 12 * HD], XDT,
                           kind="ExternalInput")
    wp = nc.dram_tensor("wp", [4 * HD, C], F32R, kind="ExternalInput")
    cos2_d = nc.dram_tensor("cos2", [128, T], F32, kind="ExternalInput")
    sin2_d = nc.dram_tensor("sin2", [128, T], F32, kind="ExternalInput")
    rot_d = nc.dram_tensor("rot", [128, 128], F32R, kind="ExternalInput")
    cmask_d = nc.dram_tensor("cmask", [128, 256], BF16, kind="ExternalInput")
    outp = nc.dram_tensor("outp", [T, C], F32, kind="ExternalOutput")

    NT = T // 128    # 16 k-tiles
    NQ = T // 512    # 4 q-chunks
    NWARM = int(os.environ.get("KERNEL_NWARM", "24"))
    PIPE = int(os.environ.get("KERNEL_PIPE", "3"))
    PROJ_LAG = int(os.environ.get("KERNEL_PROJLAG", "2"))
    ADD_ENG = os.environ.get("KERNEL_ADDE", "pool")
    NORM_ENG = os.environ.get("KERNEL_NORME", "pool")

    with tile.TileContext(nc) as tc:
        with (
            tc.tile_pool(name="persist", bufs=1) as persist,
            tc.tile_pool(name="consts", bufs=1) as consts,
            tc.tile_pool(name="psall", bufs=4, space="PSUM") as psall,
        ):
            # ---- persistent tiles (across phases) ----
            qk_packed = [
                persist.tile([128, T], F32R, name=f"qkp{w}", tag=f"qkp{w}")
                for w in range(4)
            ]
            vtil = persist.tile([128, NT, 4, HD + 1], EDT, name="vtil")
            ynorm = [
                persist.tile([128, T], F32R, name=f"ynorm{g}", tag=f"ynorm{g}")
                for g in range(2)
            ]
            cmask_sb = consts.tile([128, 256], BF16, name="cmask_sb")
            nc.sync.dma_start(out=cmask_sb, in_=cmask_d[:, :])
            ident_sb = cmask_sb[:, 0:128]
            masktri_sb = cmask_sb[:, 128:256]
            wp_sb = consts.tile([128, 2, C], F32R, name="wp_sb")

            # ---- PE warmup: ramp the clock while input DMAs land ----
            if NWARM:
                ps_warm = psall.tile([128, 2, 512], F32, name="ps_warm",
                                     tag="psS", bufs=2)
                for i in range(NWARM):
                    nc.tensor.matmul(ps_warm[:, 0, 0:256], lhsT=ident_sb,
                                     rhs=cmask_sb, start=True, stop=True)

            # ================= Phase 1: QKV + RoPE =================
            with (
                tc.tile_pool(name="p1", bufs=1) as p1,
                tc.tile_pool(name="p1tmp", bufs=1) as p1tmp,
            ):
                xT_sb = p1.tile([128, C // 128, T], XDT, name="xT_sb")
                w_sb = p1.tile([128, C // 128, 12 * HD], XDT, name="w_sb")
                xT_r = xT.rearrange("(co p) t -> p co t", p=128)
                # DMA emission order = first-consumption order.
                nc.sync.dma_start(out=w_sb[:, 0:2, :], in_=w_all[:, 0:2, :])
                nc.sync.dma_start(out=xT_sb[:, :, 0:512], in_=xT_r[:, :, 0:512])
                nc.sync.dma_start(out=w_sb[:, 2:8, :], in_=w_all[:, 2:8, :])
                rot_sb = p1.tile([128, 128], F32R, name="rot_sb")
                nc.sync.dma_start(out=rot_sb, in_=rot_d[:, :])
                cos2_sb = p1.tile([128, T], F32, name="cos2_sb")
                sin2_sb = p1.tile([128, T], F32, name="sin2_sb")
                nc.sync.dma_start(out=cos2_sb[:, 0:512], in_=cos2_d[:, 0:512])
                nc.sync.dma_start(out=sin2_sb[:, 0:512], in_=sin2_d[:, 0:512])
                nc.sync.dma_start(out=xT_sb[:, :, 512:1024],
                                  in_=xT_r[:, :, 512:1024])
                nc.sync.dma_start(out=cos2_sb[:, 512:T], in_=cos2_d[:, 512:T])
                nc.sync.dma_start(out=sin2_sb[:, 512:T], in_=sin2_d[:, 512:T])
                nc.sync.dma_start(out=xT_sb[:, :, 1024:1536],
                                  in_=xT_r[:, :, 1024:1536])
                nc.sync.dma_start(out=xT_sb[:, :, 1536:2048],
                                  in_=xT_r[:, :, 1536:2048])
                nc.sync.dma_start(
                    out=wp_sb, in_=wp.rearrange("(gg p) n -> p gg n", p=128)
                )

                ones64 = p1.tile([128, NT * 4], F32, name="ones64")
                nc.vector.memset(ones64, 1.0)
                nc.vector.tensor_copy(
                    out=vtil[:, :, :, HD:HD + 1],
                    in_=ones64.rearrange("p (a b) -> p a b", a=NT).unsqueeze(-1),
                )

                # --- unit emitters ---
                def emit_qk_mm(w, q):
                    """8 accumulation matmuls for 128 q/k feature rows x 512
                    t-columns; returns the psum pair (qkv out, rot out)."""
                    sl = slice(q * 512, (q + 1) * 512)
                    pspair = psall.tile([128, 2, 512], F32,
                                        name="ps_qk", tag="psS", bufs=2)
                    ps = pspair[:, 0, :]
                    for c in range(C // 128):
                        nc.tensor.matmul(
                            ps,
                            lhsT=w_sb[:, c, w * 128:(w + 1) * 128],
                            rhs=xT_sb[:, c, sl],
                            start=(c == 0),
                            stop=(c == C // 128 - 1),
                        )
                    return pspair

                def emit_rot_rope(w, q, pspair, raw_eng="act"):
                    """rotate-half matmul + RoPE combine for chunk (w, q)."""
                    sl = slice(q * 512, (q + 1) * 512)
                    ps = pspair[:, 0, :]
                    raw = p1tmp.tile([128, 512], F32R, name="raw", tag="raw",
                                     bufs=2)
                    if raw_eng == "act":
                        nc.scalar.copy(out=raw, in_=ps)
                    else:
                        nc.vector.tensor_copy(out=raw, in_=ps)
                    psr = pspair[:, 1, :]
                    nc.tensor.matmul(psr, lhsT=rot_sb, rhs=raw,
                                     start=True, stop=True)
                    tmp = p1tmp.tile([128, 512], F32, name="tmp",
                                     tag="tmp", bufs=2)
                    nc.vector.tensor_mul(tmp, psr, sin2_sb[:, sl])
                    cosq = p1tmp.tile([128, 512], F32, name="cosq",
                                      tag="cosq", bufs=2)
                    nc.vector.tensor_mul(cosq, raw, cos2_sb[:, sl])
                    if ADD_ENG == "pool":
                        nc.gpsimd.tensor_add(qk_packed[w][:, sl], cosq, tmp)
                    else:
                        nc.vector.tensor_add(qk_packed[w][:, sl], cosq, tmp)

                def emit_v(tt):
                    psv = psall.tile([128, 2, 512], F32, name="psv",
                                     tag="psS", bufs=2)
                    for c in range(C // 128):
                        nc.tensor.matmul(
                            psv[:, 0, 0:4 * HD],
                            lhsT=xT_sb[:, c, tt * 128:(tt + 1) * 128],
                            rhs=w_sb[:, c, 8 * HD:12 * HD],
                            start=(c == 0),
                            stop=(c == C // 128 - 1),
                        )
                    nc.vector.tensor_copy(
                        out=vtil[:, tt, :, 0:HD],
                        in_=psv[:, 0, 0:4 * HD].rearrange(
                            "p (h d) -> p h d", h=4),
                    )

                # --- phase-1 unit stream with lag-1 rot pipeline ---
                units = [("qk", 0, q) for q in range(NQ)]
                units += [("qk", 2, q) for q in range(NQ)]
                tailu = []
                vq = [("v", tt, None) for tt in range(NT)]
                qk13 = [("qk", w, q) for w in (1, 3) for q in range(NQ)]
                # V0..V3 first (attention pair 0 needs them), then alternate
                # 2 V : 1 qk to keep the rot pipeline fed.
                tailu += vq[0:2]
                vi, qi = 2, 0
                while vi < len(vq) or qi < len(qk13):
                    if qi < len(qk13):
                        tailu.append(qk13[qi]); qi += 1
                    if vi < len(vq):
                        tailu.append(vq[vi]); vi += 1
                    if vi < len(vq):
                        tailu.append(vq[vi]); vi += 1
                units += tailu

                pend_rot = []
                nraw = 0
                for u in units:
                    kind = u[0]
                    if kind == "qk":
                        _, w, q = u
                        pspair = emit_qk_mm(w, q)
                        pend_rot.append((w, q, pspair))
                    else:
                        emit_v(u[1])
                    while len(pend_rot) > 1:
                        w0, q0, pp = pend_rot.pop(0)
                        emit_rot_rope(w0, q0, pp, raw_eng="act")
                        nraw += 1
                while pend_rot:
                    w0, q0, pp = pend_rot.pop(0)
                    emit_rot_rope(w0, q0, pp, raw_eng="act")

            # ================= Phase 2: attention =================
            with (
                tc.tile_pool(name="p2", bufs=1) as p2,
                tc.tile_pool(name="p2e",
                             bufs=int(os.environ.get("KERNEL_ESBUFS", "6"))) as p2e,
                tc.tile_pool(name="p2d",
                             bufs=int(os.environ.get("KERNEL_P2D", "2"))) as p2d,
            ):
                inv_sqrt_hd = float(1.0 / np.sqrt(HD))

                def emit_proj(tt):
                    """4 matmuls (2 output halves x 2 head-pair contractions)
                    into one 2-bank psum tile; 1 copy; 1 DMA."""
                    pso = psall.tile([128, 2, 512], F32, name="pso",
                                     tag="psS", bufs=2)
                    for nck in range(2):
                        for g in range(2):
                            nc.tensor.matmul(
                                pso[:, nck, :],
                                lhsT=ynorm[g][:, tt * 128:(tt + 1) * 128],
                                rhs=wp_sb[:, g, nck * 512:(nck + 1) * 512],
                                start=(g == 0),
                                stop=(g == 1),
                            )
                    ost = p2e.tile([128, 2, 512], F32, name="ost", tag="eS0")
                    nc.vector.tensor_copy(out=ost, in_=pso)
                    nc.sync.dma_start(
                        out=outp[tt * 128:(tt + 1) * 128, :],
                        in_=ost.rearrange("p a b -> p (a b)"),
                    )

                proj_queue = []

                for g in range(2):          # head-pair (pack) index
                    ytils = []
                    for hh in range(2):
                        ytils.append(
                            p2.tile([HD + 1, NQ, 512], F32, name=f"ytil{g}{hh}",
                                    tag=f"ytil{hh}")
                        )

                    def make_psY():
                        return [
                            psall.tile([HD + 1, 512], F32, name=f"psY{hh}",
                                       tag=f"yacc{hh}", bufs=2,
                                       padded_shape=[128, 512])
                            for hh in range(2)
                        ]

                    def emit_S(cq, j):
                        """S^T matmuls for both heads of the pair at k-tile j,
                        trimmed to the causally-valid column suffix; returns
                        the exp'd tiles + offset."""
                        off = max(0, (j - 4 * cq) * 128)
                        F = 512 - off
                        qlo = cq * 512 + off
                        psS = psall.tile([128, 2, 512], F32, name="psS",
                                         tag="psS", bufs=2)
                        diag = j >= 4 * cq
                        for hh in range(2):
                            poff = 64 * hh
                            nc.tensor.matmul(
                                psS[:, hh, 0:F],
                                lhsT=qk_packed[2 + g][
                                    poff:poff + 64, j * 128:(j + 1) * 128],
                                rhs=qk_packed[g][
                                    poff:poff + 64, qlo:qlo + F],
                                start=True,
                                stop=not diag,
                            )
                            if diag:
                                # the causal triangle only ever occupies the
                                # first 128 columns of the trimmed block
                                nc.tensor.matmul(
                                    psS[:, hh, 0:128],
                                    lhsT=ident_sb,
                                    rhs=masktri_sb,
                                    start=False,
                                    stop=True,
                                )
                        eSp = p2e.tile([128, 2, 512], EDT, name="eSp",
                                       tag="eSp")
                        nc.scalar.activation(
                            out=eSp[:, :, 0:F], in_=psS[:, :, 0:F],
                            func=mybir.ActivationFunctionType.Exp,
                            scale=inv_sqrt_hd,
                        )
                        es = [eSp[:, 0, :], eSp[:, 1, :]]
                        return es, off

                    def emit_PV(cq, j, es_off, psY):
                        es, off = es_off
                        njt = 4 * cq + 4
                        F = 512 - off
                        for hh in range(2):
                            h = 2 * g + hh      # local head in 0..3
                            nc.tensor.matmul(
                                psY[hh][:, off:512],
                                lhsT=vtil[:, j, h, :],
                                rhs=es[hh][:, 0:F],
                                start=(j == 0),
                                stop=(j == njt - 1),
                            )

                    def finish_chunk(cq, psY):
                        for hh in range(2):
                            # numerators + denominator row -> SBUF
                            nc.vector.tensor_copy(
                                out=ytils[hh][:, cq, :],
                                in_=psY[hh][:, :],
                            )
                            # reciprocal of the denominator row, broadcast to
                            # 64 partitions on the idle GPSIMD, then multiply
                            rec = p2d.tile([1, 512], F32, name="rec",
                                           tag="rec")
                            nc.vector.reciprocal(
                                rec, ytils[hh][HD:HD + 1, cq, :])
                            bc64 = p2d.tile([64, 512], F32, name="bc64",
                                            tag="bc64")
                            nc.gpsimd.partition_broadcast(bc64, rec)
                            neng = (nc.gpsimd if NORM_ENG == "pool"
                                    else nc.vector)
                            if hh == 0:
                                neng.tensor_mul(
                                    ynorm[g][0:64, cq * 512:(cq + 1) * 512],
                                    ytils[hh][0:64, cq, :],
                                    bc64,
                                )
                            else:
                                fix = p2d.tile([64, 512], F32R, name="fix",
                                               tag="fix")
                                neng.tensor_mul(
                                    fix,
                                    ytils[hh][0:64, cq, :],
                                    bc64,
                                )
                                nc.sync.dma_start(
                                    out=ynorm[g][64:128,
                                                 cq * 512:(cq + 1) * 512],
                                    in_=fix,
                                )

                    # flattened (cq, j) stream: the S->exp->PV pipeline
                    # carries across chunk boundaries so it never drains
                    steps = [(cq, j) for cq in range(NQ)
                             for j in range(4 * cq + 4)]
                    psYs = {}
                    pend = []

                    def pop_one():
                        (pcq, pj), es = pend.pop(0)
                        emit_PV(pcq, pj, es, psYs[pcq])
                        if pj == 4 * pcq + 3:       # last k-tile of chunk
                            finish_chunk(pcq, psYs.pop(pcq))
                            if g == 1:
                                proj_queue.extend(
                                    range(4 * pcq, 4 * pcq + 4))

                    nsteps = 0
                    for (cq, j) in steps:
                        if cq not in psYs:
                            psYs[cq] = make_psY()
                        pend.append(((cq, j), emit_S(cq, j)))
                        if len(pend) > PIPE:
                            pop_one()
                        nsteps += 1
                        # interleave projection with a lag so its matmuls
                        # never head-block the PE queue waiting on ynorm
                        if g == 1 and proj_queue and nsteps % 2 == 0:
                            emit_proj(proj_queue.pop(0))
                    while pend:
                        pop_one()
                    if g == 1:
                        while proj_queue:
                            emit_proj(proj_queue.pop(0))

    nc.finalize()
    return nc


def _prep_in_maps(x, w_attn, w_proj):
    import ml_dtypes
    xdt = ml_dtypes.bfloat16 if XDT_BF16 else np.float32
    x = np.asarray(x, dtype=np.float32)
    w_attn = np.asarray(w_attn, dtype=np.float32)
    w_proj = np.asarray(w_proj, dtype=np.float32)

    cos2, sin2 = _rope_tables_T()
    rot = _rot_matrix()
    cmask = _cmask_tile()

    xTs = [np.ascontiguousarray(x[b].T.astype(xdt)) for b in range(B)]
    in_maps = []
    for core in range(N_CORES):
        b = core // 4
        hbase = (core % 4) * HPC
        # w_all columns: [q_h0|q_h1, q_h2|q_h3, k_h0|k_h1, k_h2|k_h3, v x4]
        qcols = w_attn[:, hbase * HD:(hbase + HPC) * HD]
        kcols = w_attn[:, C + hbase * HD:C + (hbase + HPC) * HD]
        vcols = w_attn[:, 2 * C + hbase * HD:2 * C + (hbase + HPC) * HD]
        w_cat = np.concatenate([qcols, kcols, vcols], axis=1)   # [C, 768]
        # pre-swizzle to [p, co, n] so the device DMA is descriptor-dense
        w_swz = np.ascontiguousarray(
            w_cat.reshape(C // 128, 128, 12 * HD).transpose(1, 0, 2)
        ).astype(xdt)
        wp_ = np.ascontiguousarray(w_proj[hbase * HD:(hbase + HPC) * HD, :])
        in_maps.append({
            "xT": xTs[b],
            "w_all": w_swz,
            "wp": wp_,
            "cos2": cos2,
            "sin2": sin2,
            "rot": rot,
            "cmask": cmask,
        })
    return in_maps


def _get_runner():
    """Build the SPMD jitted callable once and cache it (mirrors
    bass2jax.run_bass_via_pjrt, but reusable across kernel() calls)."""
    if "runner" in _CACHE:
        return _CACHE["runner"]

    import jax
    from jax.sharding import Mesh, PartitionSpec
    try:
        from jax.experimental.shard_map import shard_map
    except ImportError:
        from jax.shard_map import shard_map  # newer jax
    import concourse.mybir as _mybir
    from concourse import bass2jax

    nc = build_nc()
    _CACHE["nc"] = nc
    bass2jax.install_neuronx_cc_hook()

    partition_name = (
        nc.partition_id_tensor.name if nc.partition_id_tensor else None
    )
    in_names, out_names, out_avals, zero_outs = [], [], [], []
    for alloc in nc.m.functions[0].allocations:
        if not isinstance(alloc, _mybir.MemoryLocationSet):
            continue
        name = alloc.memorylocations[0].name
        if alloc.kind == "ExternalInput":
            if name != partition_name:
                in_names.append(name)
        elif alloc.kind == "ExternalOutput":
            shape = tuple(alloc.tensor_shape)
            dtype = _mybir.dt.np(alloc.dtype)
            out_names.append(name)
            out_avals.append(jax.core.ShapedArray(shape, dtype))
            zero_outs.append(np.zeros(shape, dtype))
    n_params = len(in_names)
    all_names = list(in_names) + list(out_names)
    if partition_name is not None:
        all_names.append(partition_name)
    donate = tuple(range(n_params, n_params + len(out_names)))

    def _body(*args):
        operands = list(args)
        if partition_name is not None:
            operands.append(bass2jax.partition_id_tensor())
        outs = bass2jax._bass_exec_p.bind(
            *operands,
            out_avals=tuple(out_avals),
            in_names=tuple(all_names),
            out_names=tuple(out_names),
            lowering_input_output_aliases=(),
            sim_require_finite=True,
            sim_require_nnan=True,
            nc=nc,
        )
        return tuple(outs)

    devices = jax.devices()[:N_CORES]
    mesh = Mesh(np.asarray(devices), ("core",))
    in_specs = (PartitionSpec("core"),) * (n_params + len(out_names))
    out_specs = (PartitionSpec("core"),) * len(out_names)
    sharded = jax.jit(
        shard_map(_body, mesh=mesh, in_specs=in_specs, out_specs=out_specs,
                  check_rep=False),
        donate_argnums=donate,
        keep_unused=True,
    )

    def run(in_maps):
        concat_in = [
            np.concatenate([np.asarray(in_maps[c][nm]) for c in range(N_CORES)],
                           axis=0)
            for nm in in_names
        ]
        concat_zeros = [
            np.zeros((N_CORES * z.shape[0], *z.shape[1:]), z.dtype)
            for z in zero_outs
        ]
        out_arrs = sharded(*concat_in, *concat_zeros)
        return [
            {
                nm: np.asarray(out_arrs[i]).reshape(
                    N_CORES, *out_avals[i].shape)[c]
                for i, nm in enumerate(out_names)
            }
            for c in range(N_CORES)
        ]

    _CACHE["runner"] = run
    return run


def kernel(x, w_attn, w_proj, n_head):
    assert int(n_head) == NH
    x = np.asarray(x, dtype=np.float32)
    assert x.shape == (B, T, C), x.shape

    in_maps = _prep_in_maps(x, np.asarray(w_attn), np.asarray(w_proj))
    if _CACHE.get("use_fallback"):
        results = _run_fallback(in_maps)
    else:
        try:
            run = _get_runner()
            results = run(in_maps)
        except Exception:
            _CACHE["use_fallback"] = True
            results = _run_fallback(in_maps)
    out = np.zeros((B, T, C), dtype=np.float32)
    for core in range(N_CORES):
        out[core // 4] += results[core]["outp"]
    return out


def _run_fallback(in_maps):
    """Native-NRT path (run_bass_kernel_spmd) for non-axon hosts."""
    if "nc" not in _CACHE:
        _CACHE["nc"] = build_nc()
    res = run_bass_kernel_spmd(_CACHE["nc"], in_maps,
                               core_ids=list(range(N_CORES)))
    return res.results


if __name__ == "__main__":
    rng = np.random.default_rng(0)
    x = rng.standard_normal((B, T, C)).astype(np.float32)
    wa = (rng.standard_normal((C, 3 * C)) / np.sqrt(C)).astype(np.float32)
    wpj = (rng.standard_normal((C, C)) / np.sqrt(C)).astype(np.float32)
    y = kernel(x, wa, wpj, NH)
    print("kernel ran, out:", y.shape, y.dtype, float(np.abs(y).mean()))


# revision 10
# speedup vs baseline: 1.0686x; 1.0686x over previous
"""Causal self-attention with RoPE on 8 Trainium2 NeuronCores (Bass/Tile).

Sharding: 8 cores = 2 batch elements x 4 head-groups (4 heads each), no
collectives. Each core computes QKV for its heads from a host-pretransposed
x^T, applies RoPE, runs causally-trimmed flash-style attention, and emits a
partial output projection against its w_proj row-slice; the host sums 4
partials per batch element.

Key design points:
- bf16 x / qkv weights (full PE rate, ~1e-3 end-to-end error); fp32r
  elsewhere.
- Consolidated DMAs: one descriptor-dense DMA per x^T q-chunk, one merged
  [qkv] weight tensor, whole-table cos/sin loads. The cost model charges
  ~625ns of exclusive HWDGE time per DMA, so DMA count matters.
- PE warmup matmuls on the mask constants ramp the tensor engine to full
  clock while the first real DMAs land.
- Zero on-device transposes: x^T comes from the host; Q^T/K^T are computed
  d-major (2 heads packed per 128 partitions), V is computed t-major.
- rotate_half is a PE matmul against a constant +-1 permutation; the rot
  matmul of chunk i is emitted after the QKV matmuls of chunk i+1 so the PE
  never stalls on the PSUM->SBUF copy.
- S^T blocks [k-tile, 512-wide q-chunk] are causally trimmed; the diagonal
  triangle mask (only ever 128 columns wide) is accumulated into PSUM by a
  bf16 identity-matmul after the S matmul.
- exp on ScalarE reads PSUM directly (scale=1/sqrt(hd) folded in); V~ has a
  ones column so the PV matmul accumulates the softmax denominator for free.
- Normalization: single-partition reciprocal + gpsimd partition_broadcast
  (no DMAs in the chain); the combine add and normalize muls run on the
  otherwise-idle GPSIMD where possible.
- Projection is interleaved into pair-1 attention with a lag so its matmuls
  never head-block the PE queue.
"""
import os

import numpy as np

import concourse.bass as bass
import concourse.mybir as mybir
import concourse.tile as tile
from concourse import bacc
from concourse.bass_utils import run_bass_kernel_spmd

# Problem shape (hardcoded per harness contract).
B, T, C, NH = 2, 2048, 1024, 16
HD = C // NH          # 64
HPC = NH // 4         # 4 heads per core
N_CORES = 8
ROPE_BASE = 10000.0
NEG = -1.0e30

F32 = mybir.dt.float32
BF16 = mybir.dt.bfloat16
USE_F32R = os.environ.get("KERNEL_F32R", "1") == "1"
F32R = mybir.dt.float32r if USE_F32R else mybir.dt.float32
EDT_BF16 = os.environ.get("KERNEL_EDT", "bf16") == "bf16"
XDT_BF16 = os.environ.get("KERNEL_XDT", "bf16") == "bf16"

_CACHE = {}


def _rope_tables_T():
    """cos/sin tables transposed to [HD, T], duplicated to 128 partitions
    (two 64-row head blocks), with the rotate-half sign folded into sin."""
    inv_freq = 1.0 / (ROPE_BASE ** (np.arange(0, HD, 2, dtype=np.float32) / HD))
    t = np.arange(T, dtype=np.float32)
    freqs = np.outer(t, inv_freq).astype(np.float32)      # [T, 32]
    emb = np.concatenate([freqs, freqs], axis=-1)         # [T, 64]
    cosT = np.cos(emb).T.astype(np.float32)               # [64, T]
    sinT = np.sin(emb).T.astype(np.float32)
    cos2 = np.concatenate([cosT, cosT], axis=0)           # [128, T]
    sin2 = np.concatenate([sinT, sinT], axis=0)
    return np.ascontiguousarray(cos2), np.ascontiguousarray(sin2)


def _rot_matrix():
    """rot128 so that (rot128.T @ qT) = rotate_half(q)^T per 64-row head
    block: out[d] = -in[d+32] for d<32, in[d-32] for d>=32."""
    r = np.zeros((64, 64), dtype=np.float32)
    for d in range(32):
        r[d, d + 32] = -1.0
        r[d + 32, d] = 1.0
    z = np.zeros_like(r)
    rot = np.block([[r, z], [z, r]])          # [128, 128]
    return np.ascontiguousarray(rot.T)        # lhsT layout



def _cmask_tile():
    """[128, 256] bf16: identity in cols 0-127, causal triangle (0 / NEG)
    in cols 128-255."""
    import ml_dtypes
    ident = np.eye(128, dtype=np.float32)
    k_l = np.arange(128)[:, None]
    q_l = np.arange(128)[None, :]
    tri = np.where(q_l >= k_l, 0.0, NEG)
    return np.concatenate([ident, tri], axis=1).astype(ml_dtypes.bfloat16)


def build_nc():
    nc = bacc.Bacc(None, target_bir_lowering=False)

    XDT = BF16 if XDT_BF16 else F32R
    EDT = BF16 if EDT_BF16 else F32R
    xT = nc.dram_tensor("xT", [C, T], XDT, kind="ExternalInput")
    # weights stay f32r: f32r lhsT is self-loading while a bf16 lhsT costs a
    # standalone Ldweights instruction per matmul
    w_all = nc.dram_tensor("w_all", [128, C // 128, 12 * HD], F32R,
                           kind="ExternalInput")
    wp = nc.dram_tensor("wp", [4 * HD, C], F32R, kind="ExternalInput")
    cos2_d = nc.dram_tensor("cos2", [128, T], F32, kind="ExternalInput")
    sin2_d = nc.dram_tensor("sin2", [128, T], F32, kind="ExternalInput")
    rot_d = nc.dram_tensor("rot", [128, 128], F32R, kind="ExternalInput")
    cmask_d = nc.dram_tensor("cmask", [128, 256], BF16, kind="ExternalInput")
    outp = nc.dram_tensor("outp", [T, C], F32, kind="ExternalOutput")

    NT = T // 128    # 16 k-tiles
    NQ = T // 512    # 4 q-chunks
    NWARM = int(os.environ.get("KERNEL_NWARM", "24"))
    PIPE = int(os.environ.get("KERNEL_PIPE", "3"))
    PROJ_LAG = int(os.environ.get("KERNEL_PROJLAG", "2"))
    ADD_ENG = os.environ.get("KERNEL_ADDE", "pool")
    NORM_ENG = os.environ.get("KERNEL_NORME", "pool")

    with tile.TileContext(nc) as tc:
        with (
            tc.tile_pool(name="persist", bufs=1) as persist,
            tc.tile_pool(name="consts", bufs=1) as consts,
            tc.tile_pool(name="psall", bufs=4, space="PSUM") as psall,
        ):
            # ---- persistent tiles (across phases) ----
            qk_packed = [
                persist.tile([128, T], F32R, name=f"qkp{w}", tag=f"qkp{w}")
                for w in range(4)
            ]
            vtil = persist.tile([128, NT, 4, HD + 1], F32R, name="vtil")
            ynorm = [
                persist.tile([128, T], F32R, name=f"ynorm{g}", tag=f"ynorm{g}")
                for g in range(2)
            ]
            cmask_sb = consts.tile([128, 256], BF16, name="cmask_sb")
            nc.sync.dma_start(out=cmask_sb, in_=cmask_d[:, :])
            ident_sb = cmask_sb[:, 0:128]
            masktri_sb = cmask_sb[:, 128:256]
            wp_sb = consts.tile([128, 2, C], F32R, name="wp_sb")

            # ---- PE warmup: ramp the clock while input DMAs land ----
            if NWARM:
                ps_warm = psall.tile([128, 2, 512], F32, name="ps_warm",
                                     tag="psS", bufs=2)
                for i in range(NWARM):
                    nc.tensor.matmul(ps_warm[:, 0, 0:256], lhsT=ident_sb,
                                     rhs=cmask_sb, start=True, stop=True)

            # ================= Phase 1: QKV + RoPE =================
            with (
                tc.tile_pool(name="p1", bufs=1) as p1,
                tc.tile_pool(name="p1tmp", bufs=1) as p1tmp,
            ):
                xT_sb = p1.tile([128, C // 128, T], XDT, name="xT_sb")
                w_sb = p1.tile([128, C // 128, 12 * HD], F32R, name="w_sb")
                xT_r = xT.rearrange("(co p) t -> p co t", p=128)
                # DMA emission order = first-consumption order.
                nc.sync.dma_start(out=w_sb[:, 0:2, :], in_=w_all[:, 0:2, :])
                nc.sync.dma_start(out=xT_sb[:, :, 0:512], in_=xT_r[:, :, 0:512])
                nc.sync.dma_start(out=w_sb[:, 2:8, :], in_=w_all[:, 2:8, :])
                rot_sb = p1.tile([128, 128], F32R, name="rot_sb")
                nc.sync.dma_start(out=rot_sb, in_=rot_d[:, :])
                cos2_sb = p1.tile([128, T], F32, name="cos2_sb")
                sin2_sb = p1.tile([128, T], F32, name="sin2_sb")
                nc.sync.dma_start(out=cos2_sb[:, 0:512], in_=cos2_d[:, 0:512])
                nc.sync.dma_start(out=sin2_sb[:, 0:512], in_=sin2_d[:, 0:512])
                nc.sync.dma_start(out=xT_sb[:, :, 512:1024],
                                  in_=xT_r[:, :, 512:1024])
                nc.sync.dma_start(out=xT_sb[:, :, 1024:1536],
                                  in_=xT_r[:, :, 1024:1536])
                nc.sync.dma_start(out=cos2_sb[:, 512:T], in_=cos2_d[:, 512:T])
                nc.sync.dma_start(out=sin2_sb[:, 512:T], in_=sin2_d[:, 512:T])
                nc.sync.dma_start(out=xT_sb[:, :, 1536:2048],
                                  in_=xT_r[:, :, 1536:2048])
                nc.sync.dma_start(
                    out=wp_sb, in_=wp.rearrange("(gg p) n -> p gg n", p=128)
                )

                ones64 = p1.tile([128, NT * 4], F32, name="ones64")
                nc.vector.memset(ones64, 1.0)
                nc.vector.tensor_copy(
                    out=vtil[:, :, :, HD:HD + 1],
                    in_=ones64.rearrange("p (a b) -> p a b", a=NT).unsqueeze(-1),
                )

                # --- unit emitters ---
                def emit_qk_mm(w, q):
                    """8 accumulation matmuls for 128 q/k feature rows x 512
                    t-columns; returns the psum pair (qkv out, rot out).
                    The qk psums live on the yacc tags, which attention's psY
                    will only start using after phase 1 drains."""
                    sl = slice(q * 512, (q + 1) * 512)
                    ps = psall.tile([128, 512], F32, name="ps_qk",
                                    tag="yacc0", bufs=2,
                                    padded_shape=[128, 512])
                    for c in range(C // 128):
                        nc.tensor.matmul(
                            ps,
                            lhsT=w_sb[:, c, w * 128:(w + 1) * 128],
                            rhs=xT_sb[:, c, sl],
                            start=(c == 0),
                            stop=(c == C // 128 - 1),
                        )
                    return ps

                def emit_rot_rope(w, q, ps, raw_eng="act"):
                    """rotate-half matmul + RoPE combine for chunk (w, q)."""
                    sl = slice(q * 512, (q + 1) * 512)
                    raw = p1tmp.tile([128, 512], F32R, name="raw", tag="raw",
                                     bufs=2)
                    if raw_eng == "act":
                        nc.scalar.copy(out=raw, in_=ps)
                    else:
                        nc.vector.tensor_copy(out=raw, in_=ps)
                    psr = psall.tile([128, 512], F32, name="psr",
                                     tag="yacc1", bufs=2,
                                     padded_shape=[128, 512])
                    nc.tensor.matmul(psr, lhsT=rot_sb, rhs=raw,
                                     start=True, stop=True)
                    tmp = p1tmp.tile([128, 512], F32, name="tmp",
                                     tag="tmp", bufs=2)
                    nc.vector.tensor_mul(tmp, psr, sin2_sb[:, sl])
                    cosq = p1tmp.tile([128, 512], F32, name="cosq",
                                      tag="cosq", bufs=2)
                    nc.vector.tensor_mul(cosq, raw, cos2_sb[:, sl])
                    if ADD_ENG == "pool":
                        nc.gpsimd.tensor_add(qk_packed[w][:, sl], cosq, tmp)
                    else:
                        nc.vector.tensor_add(qk_packed[w][:, sl], cosq, tmp)

                def emit_v(tt):
                    psv = psall.tile([128, 2, 512], F32, name="psv",
                                     tag="psS", bufs=2)
                    for c in range(C // 128):
                        nc.tensor.matmul(
                            psv[:, 0, 0:4 * HD],
                            lhsT=xT_sb[:, c, tt * 128:(tt + 1) * 128],
                            rhs=w_sb[:, c, 8 * HD:12 * HD],
                            start=(c == 0),
                            stop=(c == C // 128 - 1),
                        )
                    nc.vector.tensor_copy(
                        out=vtil[:, tt, :, 0:HD],
                        in_=psv[:, 0, 0:4 * HD].rearrange(
                            "p (h d) -> p h d", h=4),
                    )

                # --- phase-1 unit stream, q-major (matches xT DMA arrival
                # order), with the rot pipeline one unit behind ---
                units = []
                for q in range(NQ):
                    units += [("qk", 0, q), ("qk", 2, q)]
                    units += [("v", tt, None) for tt in range(4 * q, 4 * q + 4)]
                for q in range(NQ):
                    units += [("qk", 1, q), ("qk", 3, q)]

                pend_rot = []
                for u in units:
                    kind = u[0]
                    if kind == "qk":
                        _, w, q = u
                        ps = emit_qk_mm(w, q)
                        pend_rot.append((w, q, ps))
                    else:
                        emit_v(u[1])
                    while len(pend_rot) > 1:
                        w0, q0, pp = pend_rot.pop(0)
                        emit_rot_rope(w0, q0, pp, raw_eng="act")
                while pend_rot:
                    w0, q0, pp = pend_rot.pop(0)
                    emit_rot_rope(w0, q0, pp, raw_eng="act")

            # ================= Phase 2: attention =================
            with (
                tc.tile_pool(name="p2", bufs=1) as p2,
                tc.tile_pool(name="p2e",
                             bufs=int(os.environ.get("KERNEL_ESBUFS", "6"))) as p2e,
                tc.tile_pool(name="p2d",
                             bufs=int(os.environ.get("KERNEL_P2D", "2"))) as p2d,
            ):
                inv_sqrt_hd = float(1.0 / np.sqrt(HD))

                def emit_proj(tt):
                    """4 matmuls (2 output halves x 2 head-pair contractions)
                    into one 2-bank psum tile; 1 copy; 1 DMA."""
                    pso = psall.tile([128, 2, 512], F32, name="pso",
                                     tag="psS", bufs=2)
                    for nck in range(2):
                        for g in range(2):
                            nc.tensor.matmul(
                                pso[:, nck, :],
                                lhsT=ynorm[g][:, tt * 128:(tt + 1) * 128],
                                rhs=wp_sb[:, g, nck * 512:(nck + 1) * 512],
                                start=(g == 0),
                                stop=(g == 1),
                            )
                    ost = p2e.tile([128, 2, 512], F32, name="ost", tag="eS0")
                    nc.vector.tensor_copy(out=ost, in_=pso)
                    nc.sync.dma_start(
                        out=outp[tt * 128:(tt + 1) * 128, :],
                        in_=ost.rearrange("p a b -> p (a b)"),
                    )

                proj_queue = []

                for g in range(2):          # head-pair (pack) index
                    ytils = []
                    for hh in range(2):
                        ytils.append(
                            p2.tile([HD + 1, NQ, 512], F32, name=f"ytil{g}{hh}",
                                    tag=f"ytil{hh}")
                        )

                    def make_psY():
                        return [
                            psall.tile([HD + 1, 512], F32, name=f"psY{hh}",
                                       tag=f"yacc{hh}", bufs=2,
                                       padded_shape=[128, 512])
                            for hh in range(2)
                        ]

                    def emit_S(cq, j):
                        """S^T matmuls for both heads of the pair at k-tile j,
                        trimmed to the causally-valid column suffix; returns
                        the exp'd tiles + offset."""
                        off = max(0, (j - 4 * cq) * 128)
                        F = 512 - off
                        qlo = cq * 512 + off
                        psS = psall.tile([128, 2, 512], F32, name="psS",
                                         tag="psS", bufs=2)
                        diag = j >= 4 * cq
                        for hh in range(2):
                            poff = 64 * hh
                            nc.tensor.matmul(
                                psS[:, hh, 0:F],
                                lhsT=qk_packed[2 + g][
                                    poff:poff + 64, j * 128:(j + 1) * 128],
                                rhs=qk_packed[g][
                                    poff:poff + 64, qlo:qlo + F],
                                start=True,
                                stop=not diag,
                            )
                            if diag:
                                # the causal triangle only ever occupies the
                                # first 128 columns of the trimmed block
                                nc.tensor.matmul(
                                    psS[:, hh, 0:128],
                                    lhsT=ident_sb,
                                    rhs=masktri_sb,
                                    start=False,
                                    stop=True,
                                )
                        eSp = p2e.tile([128, 2, 512], EDT, name="eSp",
                                       tag="eSp")
                        nc.scalar.activation(
                            out=eSp[:, :, 0:F], in_=psS[:, :, 0:F],
                            func=mybir.ActivationFunctionType.Exp,
                            scale=inv_sqrt_hd,
                        )
                        es = [eSp[:, 0, :], eSp[:, 1, :]]
                        return es, off

                    def emit_PV(cq, j, es_off, psY):
                        es, off = es_off
                        njt = 4 * cq + 4
                        F = 512 - off
                        for hh in range(2):
                            h = 2 * g + hh      # local head in 0..3
                            nc.tensor.matmul(
                                psY[hh][:, off:512],
                                lhsT=vtil[:, j, h, :],
                                rhs=es[hh][:, 0:F],
                                start=(j == 0),
                                stop=(j == njt - 1),
                            )

                    def finish_chunk(cq, psY):
                        for hh in range(2):
                            # numerators + denominator row -> SBUF
                            nc.vector.tensor_copy(
                                out=ytils[hh][:, cq, :],
                                in_=psY[hh][:, :],
                            )
                            # reciprocal of the denominator row, broadcast to
                            # 64 partitions on the idle GPSIMD, then multiply
                            rec = p2d.tile([1, 512], F32, name="rec",
                                           tag="rec")
                            nc.vector.reciprocal(
                                rec, ytils[hh][HD:HD + 1, cq, :])
                            bc64 = p2d.tile([64, 512], F32, name="bc64",
                                            tag="bc64")
                            nc.gpsimd.partition_broadcast(bc64, rec)
                            neng = (nc.gpsimd if NORM_ENG == "pool"
                                    else nc.vector)
                            if hh == 0:
                                neng.tensor_mul(
                                    ynorm[g][0:64, cq * 512:(cq + 1) * 512],
                                    ytils[hh][0:64, cq, :],
                                    bc64,
                                )
                            else:
                                fix = p2d.tile([64, 512], F32R, name="fix",
                                               tag="fix")
                                neng.tensor_mul(
                                    fix,
                                    ytils[hh][0:64, cq, :],
                                    bc64,
                                )
                                nc.sync.dma_start(
                                    out=ynorm[g][64:128,
                                                 cq * 512:(cq + 1) * 512],
                                    in_=fix,
                                )

                    # flattened (cq, j) stream: the S->exp->PV pipeline
                    # carries across chunk boundaries so it never drains
                    steps = [(cq, j) for cq in range(NQ)
                             for j in range(4 * cq + 4)]
                    psYs = {}
                    pend = []

                    def pop_one():
                        (pcq, pj), es = pend.pop(0)
                        emit_PV(pcq, pj, es, psYs[pcq])
                        if pj == 4 * pcq + 3:       # last k-tile of chunk
                            finish_chunk(pcq, psYs.pop(pcq))
                            if g == 1:
                                proj_queue.extend(
                                    range(4 * pcq, 4 * pcq + 4))

                    nsteps = 0
                    for (cq, j) in steps:
                        if cq not in psYs:
                            psYs[cq] = make_psY()
                        pend.append(((cq, j), emit_S(cq, j)))
                        if len(pend) > PIPE:
                            pop_one()
                        nsteps += 1
                        # interleave projection with a lag so its matmuls
                        # never head-block the PE queue waiting on ynorm
                        if g == 1 and proj_queue and nsteps % 2 == 0:
                            emit_proj(proj_queue.pop(0))
                    while pend:
                        pop_one()
                    if g == 1:
                        while proj_queue:
                            emit_proj(proj_queue.pop(0))

    nc.finalize()
    return nc


def _prep_in_maps(x, w_attn, w_proj):
    import ml_dtypes
    xdt = ml_dtypes.bfloat16 if XDT_BF16 else np.float32
    x = np.asarray(x, dtype=np.float32)
    w_attn = np.asarray(w_attn, dtype=np.float32)
    w_proj = np.asarray(w_proj, dtype=np.float32)

    cos2, sin2 = _rope_tables_T()
    rot = _rot_matrix()
    cmask = _cmask_tile()

    xTs = [np.ascontiguousarray(x[b].T.astype(xdt)) for b in range(B)]
    in_maps = []
    for core in range(N_CORES):
        b = core // 4
        hbase = (core % 4) * HPC
        # w_all columns: [q_h0|q_h1, q_h2|q_h3, k_h0|k_h1, k_h2|k_h3, v x4]
        qcols = w_attn[:, hbase * HD:(hbase + HPC) * HD]
        kcols = w_attn[:, C + hbase * HD:C + (hbase + HPC) * HD]
        vcols = w_attn[:, 2 * C + hbase * HD:2 * C + (hbase + HPC) * HD]
        w_cat = np.concatenate([qcols, kcols, vcols], axis=1)   # [C, 768]
        # pre-swizzle to [p, co, n] so the device DMA is descriptor-dense;
        # stays fp32 (f32r weights are self-loading in the PE)
        w_swz = np.ascontiguousarray(
            w_cat.reshape(C // 128, 128, 12 * HD).transpose(1, 0, 2)
        )
        wp_ = np.ascontiguousarray(w_proj[hbase * HD:(hbase + HPC) * HD, :])
        in_maps.append({
            "xT": xTs[b],
            "w_all": w_swz,
            "wp": wp_,
            "cos2": cos2,
            "sin2": sin2,
            "rot": rot,
            "cmask": cmask,
        })
    return in_maps


def _get_runner():
    """Build the SPMD jitted callable once and cache it (mirrors
    bass2jax.run_bass_via_pjrt, but reusable across kernel() calls)."""
    if "runner" in _CACHE:
        return _CACHE["runner"]

    import jax
    from jax.sharding import Mesh, PartitionSpec
    try:
        from jax.experimental.shard_map import shard_map
    except ImportError:
        from jax.shard_map import shard_map  # newer jax
    import concourse.mybir as _mybir
    from concourse import bass2jax

    nc = build_nc()
    _CACHE["nc"] = nc
    bass2jax.install_neuronx_cc_hook()

    partition_name = (
        nc.partition_id_tensor.name if nc.partition_id_tensor else None
    )
    in_names, out_names, out_avals, zero_outs = [], [], [], []
    for alloc in nc.m.functions[0].allocations:
        if not isinstance(alloc, _mybir.MemoryLocationSet):
            continue
        name = alloc.memorylocations[0].name
        if alloc.kind == "ExternalInput":
            if name != partition_name:
                in_names.append(name)
        elif alloc.kind == "ExternalOutput":
            shape = tuple(alloc.tensor_shape)
            dtype = _mybir.dt.np(alloc.dtype)
            out_names.append(name)
            out_avals.append(jax.core.ShapedArray(shape, dtype))
            zero_outs.append(np.zeros(shape, dtype))
    n_params = len(in_names)
    all_names = list(in_names) + list(out_names)
    if partition_name is not None:
        all_names.append(partition_name)
    donate = tuple(range(n_params, n_params + len(out_names)))

    def _body(*args):
        operands = list(args)
        if partition_name is not None:
            operands.append(bass2jax.partition_id_tensor())
        outs = bass2jax._bass_exec_p.bind(
            *operands,
            out_avals=tuple(out_avals),
            in_names=tuple(all_names),
            out_names=tuple(out_names),
            lowering_input_output_aliases=(),
            sim_require_finite=True,
            sim_require_nnan=True,
            nc=nc,
        )
        return tuple(outs)

    devices = jax.devices()[:N_CORES]
    mesh = Mesh(np.asarray(devices), ("core",))
    in_specs = (PartitionSpec("core"),) * (n_params + len(out_names))
    out_specs = (PartitionSpec("core"),) * len(out_names)
    sharded = jax.jit(
        shard_map(_body, mesh=mesh, in_specs=in_specs, out_specs=out_specs,
                  check_rep=False),
        donate_argnums=donate,
        keep_unused=True,
    )

    def run(in_maps):
        concat_in = [
            np.concatenate([np.asarray(in_maps[c][nm]) for c in range(N_CORES)],
                           axis=0)
            for nm in in_names
        ]
        concat_zeros = [
            np.zeros((N_CORES * z.shape[0], *z.shape[1:]), z.dtype)
            for z in zero_outs
        ]
        out_arrs = sharded(*concat_in, *concat_zeros)
        return [
            {
                nm: np.asarray(out_arrs[i]).reshape(
                    N_CORES, *out_avals[i].shape)[c]
                for i, nm in enumerate(out_names)
            }
            for c in range(N_CORES)
        ]

    _CACHE["runner"] = run
    return run


def kernel(x, w_attn, w_proj, n_head):
    assert int(n_head) == NH
    x = np.asarray(x, dtype=np.float32)
    assert x.shape == (B, T, C), x.shape

    in_maps = _prep_in_maps(x, np.asarray(w_attn), np.asarray(w_proj))
    if _CACHE.get("use_fallback"):
        results = _run_fallback(in_maps)
    else:
        try:
            run = _get_runner()
            results = run(in_maps)
        except Exception:
            _CACHE["use_fallback"] = True
            results = _run_fallback(in_maps)
    out = np.zeros((B, T, C), dtype=np.float32)
    for core in range(N_CORES):
        out[core // 4] += results[core]["outp"]
    return out


def _run_fallback(in_maps):
    """Native-NRT path (run_bass_kernel_spmd) for non-axon hosts."""
    if "nc" not in _CACHE:
        _CACHE["nc"] = build_nc()
    res = run_bass_kernel_spmd(_CACHE["nc"], in_maps,
                               core_ids=list(range(N_CORES)))
    return res.results


if __name__ == "__main__":
    rng = np.random.default_rng(0)
    x = rng.standard_normal((B, T, C)).astype(np.float32)
    wa = (rng.standard_normal((C, 3 * C)) / np.sqrt(C)).astype(np.float32)
    wpj = (rng.standard_normal((C, C)) / np.sqrt(C)).astype(np.float32)
    y = kernel(x, wa, wpj, NH)
    print("kernel ran, out:", y.shape, y.dtype, float(np.abs(y).mean()))
